# revision 2
# baseline (speedup 1.0000x reference)
import sys

sys.path.insert(0, "/opt/trn_rl_repo")

import numpy as np

# Problem constants (hardcoded per harness contract)
B = 64          # full batch
NC_CORES = 8
BPC = 8         # batches per core
N = 1024
D = 768
NS = 16         # n_slots
KT = 8          # n-tiles of 128
DT = 6          # d-tiles of 128

_CACHE = {}


def _build_nc(debug=False):
    import concourse.bacc as bacc
    import concourse.tile as tile
    import concourse.mybir as mybir
    from concourse.bass import IndirectOffsetOnAxis

    fp32 = mybir.dt.float32
    bf16 = mybir.dt.bfloat16
    i32 = mybir.dt.int32
    u32 = mybir.dt.uint32
    Alu = mybir.AluOpType
    Act = mybir.ActivationFunctionType

    nc = bacc.Bacc(
        "TRN2",
        target_bir_lowering=False,
        debug=False,
        enable_asserts=False,
        num_devices=NC_CORES,
    )

    f_dr = nc.dram_tensor("features", [BPC, N, D], fp32, kind="ExternalInput").ap()
    ident_dr = nc.dram_tensor("identity", [128, 128], fp32, kind="ExternalInput").ap()
    rowb_dr = nc.dram_tensor("rowbase", [BPC, 1], fp32, kind="ExternalInput").ap()
    out_dr = nc.dram_tensor("slots", [BPC, NS, D], fp32, kind="ExternalOutput").ap()
    g_dr = nc.dram_tensor("g_scratch", [BPC * N, N], fp32, kind="Internal").ap()
    if debug:
        dbg_sal_dr = nc.dram_tensor("dbg_sal", [BPC, N], fp32, kind="ExternalOutput").ap()
        dbg_g_dr = nc.dram_tensor("dbg_g", [128, N], fp32, kind="ExternalOutput").ap()
        dbg_idx_dr = nc.dram_tensor("dbg_idx", [BPC, NS], fp32, kind="ExternalOutput").ap()
        dbg_sim_dr = nc.dram_tensor("dbg_sim", [BPC, N], fp32, kind="ExternalOutput").ap()

    with tile.TileContext(nc) as tc:
        with (
            tc.tile_pool(name="main", bufs=1) as mp,
            tc.tile_pool(name="fbuf", bufs=2) as fbp,
            tc.tile_pool(name="fnt", bufs=1) as ftp,
            tc.tile_pool(name="gst", bufs=4) as gsp,
            tc.tile_pool(name="small", bufs=2) as smp,
            tc.tile_pool(name="psA", bufs=2, space="PSUM") as ppA,
            tc.tile_pool(name="psB", bufs=2, space="PSUM") as ppB,
        ):
            ident = mp.tile([128, 128], fp32)
            nc.sync.dma_start(ident, ident_dr)
            rowb = mp.tile([BPC, 1], fp32)
            nc.sync.dma_start(rowb, rowb_dr)

            # persistent across phases
            sal_loop = mp.tile([BPC, N], fp32)             # saliency, loop layout
            wT = mp.tile([128, KT, BPC, NS], fp32)         # slot weights, lhsT layout
            wsum = mp.tile([BPC, NS], fp32)

            # ---------------- Phase A: per-batch normalize + Gram ----------
            for b in range(BPC):
                f_sb = fbp.tile([128, KT, D], fp32, tag="f")
                nc.sync.dma_start(
                    f_sb, f_dr[b].rearrange("(kt p) d -> p kt d", p=128)
                )
                sal2 = smp.tile([128, KT], fp32, tag="sal2")
                sq_scr = smp.tile([128, D], fp32, tag="sqscr")
                for kt in range(KT):
                    nc.scalar.activation(
                        sq_scr, f_sb[:, kt], Act.Square,
                        accum_out=sal2[:, kt:kt + 1],
                    )
                salb = smp.tile([128, KT], fp32, tag="salb")
                nc.scalar.activation(salb, sal2, Act.Sqrt)
                invb = smp.tile([128, KT], fp32, tag="invb")
                nc.vector.reciprocal(invb, salb)

                # saliency into loop layout [1, N] via PE transpose
                salT_ps = ppB.tile([KT, 128], fp32, tag="tps")
                nc.tensor.transpose(salT_ps, salb, ident)
                salT = smp.tile([KT, 128], fp32, tag="salT")
                nc.scalar.copy(salT, salT_ps)
                nc.sync.dma_start(sal_loop[b:b + 1, :], salT[:, :])

                # fn (bf16 copy for slot matmuls) then scale f in place -> fn32
                for kt in range(KT):
                    nc.vector.tensor_scalar(
                        f_sb[:, kt], f_sb[:, kt], invb[:, kt:kt + 1], None,
                        op0=Alu.mult,
                    )

                # transpose fn -> fnT [128(d), DT, N]
                fnT = ftp.tile([128, DT, N], fp32, tag="fnT")
                for kt in range(KT):
                    for dt in range(DT):
                        tp = ppB.tile([128, 128], fp32, tag="tps")
                        nc.tensor.transpose(
                            tp, f_sb[:, kt, dt * 128:(dt + 1) * 128], ident
                        )
                        if (kt + dt) % 2 == 0:
                            nc.scalar.copy(
                                fnT[:, dt, kt * 128:(kt + 1) * 128], tp
                            )
                        else:
                            nc.vector.tensor_copy(
                                fnT[:, dt, kt * 128:(kt + 1) * 128], tp
                            )

                # G = fnT.T @ fnT  (normalized Gram), row tiles -> DRAM
                for i in range(KT):
                    gps = ppA.tile([128, N], fp32, tag="gps")
                    for h in range(2):
                        for dt in range(DT):
                            nc.tensor.matmul(
                                gps[:, h * 512:(h + 1) * 512],
                                fnT[:, dt, i * 128:(i + 1) * 128],
                                fnT[:, dt, h * 512:(h + 1) * 512],
                                start=(dt == 0),
                                stop=(dt == DT - 1),
                            )
                    gstage = gsp.tile([128, N], fp32, tag="gstage")
                    nc.vector.tensor_copy(gstage[:, :512], gps[:, :512])
                    nc.scalar.copy(gstage[:, 512:], gps[:, 512:])
                    nc.sync.dma_start(
                        g_dr[b * N + i * 128: b * N + (i + 1) * 128, :], gstage
                    )

            # make sure all Gram writes to DRAM are visible before gathers
            tc.strict_bb_all_engine_barrier()

            if debug:
                dbg_gt = mp.tile([128, N], fp32)
                nc.sync.dma_start(dbg_gt, g_dr[0:128, :])
                nc.sync.dma_start(dbg_g_dr, dbg_gt)
                nc.sync.dma_start(dbg_sal_dr, sal_loop)
                dbg_idx_t = mp.tile([BPC, NS], fp32)

            # ---------------- Phase B: 16-step greedy loop -----------------
            mask = mp.tile([BPC, N], fp32)
            nc.vector.memset(mask, 1.0)
            msal = mp.tile([BPC, N], fp32)
            sim = mp.tile([BPC, N], fp32)
            mx8 = mp.tile([BPC, 8], fp32)
            idx8 = mp.tile([BPC, 8], u32)
            idxf = mp.tile([BPC, 1], fp32)
            rowidx = mp.tile([BPC, 1], i32)
            w1 = mp.tile([BPC, N], fp32)
            gate = mp.tile([BPC, N], fp32)
            aggw = mp.tile([BPC, N], fp32)
            aggw_bf = mp.tile([BPC, N], bf16)
            clipv = mp.tile([BPC, N], fp32)

            sim2 = mp.tile([BPC, N], fp32)
            w1b = mp.tile([BPC, N], fp32)
            sims = [sim, sim2]
            w1s = [w1, w1b]

            def emit_deferred(t):
                # off-critical aggregation work for step t (fills gather wait)
                s = sims[t % 2]
                w = w1s[t % 2]
                nc.vector.tensor_scalar(
                    gate, s, 0.5, None, op0=Alu.is_gt
                )
                nc.vector.tensor_mul(aggw, w, gate)
                nc.scalar.activation(
                    aggw_bf, aggw, Act.Copy,
                    accum_out=wsum[:, t:t + 1],
                )
                for kt in range(KT):
                    tp2 = ppB.tile([128, 128], fp32, tag="tps")
                    nc.tensor.transpose(
                        tp2[:, :BPC],
                        aggw[:, kt * 128:(kt + 1) * 128],
                        ident[:BPC, :BPC],
                    )
                    nc.scalar.copy(wT[:, kt, :, t], tp2[:, :BPC])

            for t in range(NS):
                s = sims[t % 2]
                nc.vector.tensor_mul(msal, sal_loop, mask)
                nc.vector.max(out=mx8, in_=msal)
                nc.vector.max_index(out=idx8, in_max=mx8, in_values=msal)
                nc.vector.tensor_copy(idxf, idx8[:, 0:1])
                nc.vector.tensor_scalar(
                    rowidx, idxf, rowb, None, op0=Alu.add
                )
                if debug:
                    nc.vector.tensor_copy(dbg_idx_t[:, t:t + 1], rowidx)
                nc.gpsimd.indirect_dma_start(
                    out=s,
                    out_offset=None,
                    in_=g_dr,
                    in_offset=IndirectOffsetOnAxis(ap=rowidx, axis=0),
                )
                if t > 0:
                    emit_deferred(t - 1)
                # critical tail: uses gathered sim
                nc.vector.tensor_mul(w1s[t % 2], s, mask)
                nc.vector.tensor_scalar(
                    clipv, s, 0.0, 1.0, op0=Alu.max, op1=Alu.min
                )
                nc.vector.tensor_scalar(
                    clipv, clipv, -1.0, 1.0, op0=Alu.mult, op1=Alu.add
                )
                nc.vector.tensor_mul(mask, mask, clipv)
            emit_deferred(NS - 1)

            # ---------------- Phase C: slot matmuls ------------------------
            nc.vector.tensor_scalar(wsum, wsum, 1e-8, None, op0=Alu.add)
            recip = mp.tile([BPC, NS], fp32)
            nc.vector.reciprocal(recip, wsum)
            rT_ps = ppB.tile([128, 128], fp32, tag="tps")
            nc.tensor.transpose(rT_ps[:NS, :BPC], recip, ident[:BPC, :BPC])
            recipT = mp.tile([NS, BPC], fp32)
            nc.scalar.copy(recipT, rT_ps[:NS, :BPC])

            for b in range(BPC):
                f_c = fbp.tile([128, KT, D], fp32, tag="f")
                nc.sync.dma_start(
                    f_c, f_dr[b].rearrange("(kt p) d -> p kt d", p=128)
                )
                sp = ppA.tile([NS, D], fp32, tag="gps")
                for h, (h0, h1) in enumerate([(0, 512), (512, D)]):
                    for kt in range(KT):
                        nc.tensor.matmul(
                            sp[:, h0:h1],
                            wT[:, kt, b, :],
                            f_c[:, kt, h0:h1],
                            start=(kt == 0),
                            stop=(kt == KT - 1),
                        )
                slot_sb = gsp.tile([NS, D], fp32, tag="slot")
                nc.scalar.activation(
                    slot_sb, sp, Act.Copy, scale=recipT[:, b:b + 1]
                )
                nc.sync.dma_start(out_dr[b], slot_sb)

    nc.compile()
    return nc


def _get_nc(debug=False):
    key = ("nc", debug)
    if key not in _CACHE:
        _CACHE[key] = _build_nc(debug)
    return _CACHE[key]


def _make_in_maps(feats):
    ident = np.eye(128, dtype=np.float32)
    rowb = (np.arange(BPC, dtype=np.float32) * N).reshape(BPC, 1)
    return [
        {
            "features": feats[i * BPC:(i + 1) * BPC],
            "identity": ident,
            "rowbase": rowb,
        }
        for i in range(NC_CORES)
    ]


def kernel(features, batch_size=None, **_kw):
    from concourse import bass_utils

    nc = _get_nc()
    feats = np.ascontiguousarray(np.asarray(features, dtype=np.float32))
    in_maps = _make_in_maps(feats)
    res = bass_utils.run_bass_kernel_spmd(
        nc, in_maps, core_ids=list(range(NC_CORES))
    )
    outs = [np.asarray(res.results[i]["slots"]) for i in range(NC_CORES)]
    return np.concatenate(outs, axis=0).astype(np.float32)



# revision 19
# speedup vs baseline: 2264.9211x; 2264.9211x over previous
import sys

sys.path.insert(0, "/opt/trn_rl_repo")

import numpy as np

# Problem constants (hardcoded per harness contract)
B = 64          # full batch
NC_CORES = 8
BPC = 8         # batches per core
N = 1024
D = 768
NS = 16         # n_slots
KT = 8          # n-tiles of 128
DT = 6          # d-tiles of 128
NG = 8          # column groups per batch in blocked layout (N / 128)
BG = BPC * NG   # 64 partitions of blocked loop state

# Gram matmul dtype: "fp32" (exact, 4 cyc/row) or "fp32r" (1 cyc/row, relaxed)
import os as _os
GRAM_DT = _os.environ.get("KERNEL_GRAM_DT", "fp32")
SLOT_DT = _os.environ.get("KERNEL_SLOT_DT", "fp32")

_CACHE = {}


def _build_nc(debug=False):
    import concourse.bacc as bacc
    import concourse.tile as tile
    import concourse.mybir as mybir
    from concourse.bass import IndirectOffsetOnAxis

    fp32 = mybir.dt.float32
    fp32r = mybir.dt.float32r
    i32 = mybir.dt.int32
    u32 = mybir.dt.uint32
    u8 = mybir.dt.uint8
    Alu = mybir.AluOpType
    Act = mybir.ActivationFunctionType
    X = mybir.AxisListType.X

    gdt = fp32r if GRAM_DT == "fp32r" else fp32
    sdt = fp32r if SLOT_DT == "fp32r" else fp32

    def g_ap(ap):
        return ap.bitcast(gdt) if gdt != fp32 else ap

    def s_ap(ap):
        return ap.bitcast(sdt) if sdt != fp32 else ap

    nc = bacc.Bacc(
        "TRN2",
        target_bir_lowering=False,
        debug=False,
        enable_asserts=False,
        num_devices=NC_CORES,
    )

    f_dr = nc.dram_tensor("features", [BPC, N, D], fp32, kind="ExternalInput").ap()
    ident_dr = nc.dram_tensor("identity", [128, 128], fp32, kind="ExternalInput").ap()
    ec_dr = nc.dram_tensor("econst", [NG, BG], fp32, kind="ExternalInput").ap()
    pc_dr = nc.dram_tensor("pcol", [BG, 2], fp32, kind="ExternalInput").ap()
    cr_dr = nc.dram_tensor("crow", [1, 128], fp32, kind="ExternalInput").ap()
    out_dr = nc.dram_tensor("slots", [BPC, NS, D], fp32, kind="ExternalOutput").ap()
    # Gram scratch, viewed [BPC*N, N] for writes and [BPC*N*NG, 128] for gathers
    g_dr = nc.dram_tensor("g_scratch", [BPC * N * NG, 128], fp32, kind="Internal").ap()
    g_wr = g_dr.rearrange("(r e) c -> r (e c)", e=NG)  # [BPC*N, N]

    with tile.TileContext(nc) as tc:
        with (
            tc.tile_pool(name="main", bufs=1) as mp,
            tc.tile_pool(name="fbuf", bufs=3) as fbp,
            tc.tile_pool(name="fnt", bufs=2) as ftp,
            tc.tile_pool(name="gst", bufs=4) as gsp,
            tc.tile_pool(name="small", bufs=2) as smp,
            tc.tile_pool(name="psA", bufs=2, space="PSUM") as ppA,
            tc.tile_pool(name="psB", bufs=2, space="PSUM") as ppB,
        ):
            ident = mp.tile([128, 128], fp32)
            nc.sync.dma_start(ident, ident_dr)
            E_sb = mp.tile([NG, BG], fp32)
            nc.sync.dma_start(E_sb, ec_dr)
            pcol = mp.tile([BG, 2], fp32)
            nc.sync.dma_start(pcol, pc_dr)
            # crow: cols 0:64 = BIG (4096), cols 64:72 = b*1024
            crow = mp.tile([1, 128], fp32)
            nc.sync.dma_start(crow, cr_dr)

            # persistent loop state, blocked layout [64, 128]:
            # partition p = b*8 + g, column c; flat n = g*128 + c
            sal_blk = mp.tile([BG, 128], fp32)
            invs_blk = mp.tile([BG, 128], fp32)
            mask = mp.tile([BG, 128], fp32)
            msal = mp.tile([BG, 128], fp32)
            wT = mp.tile([128, KT, BPC, NS], fp32)   # slot weights, lhsT layout
            wsum_p = mp.tile([BG, NS], fp32)         # per-(b,g) weight sums
            # saliency + 1/saliency staged column-wise, transposed once at end
            pk_all = mp.tile([128, 2, BPC, KT], fp32)

            # ---------------- Phase A: per-batch Gram of raw features -------
            for b in range(BPC):
                f_sb = fbp.tile([128, KT, D], fp32, tag="f")
                nc.sync.dma_start(
                    f_sb, f_dr[b].rearrange("(kt p) d -> p kt d", p=128)
                )
                sal2 = smp.tile([128, KT], fp32, tag="sal2")
                sq_scr = smp.tile([128, D], fp32, tag="sqscr")
                for kt in range(KT):
                    nc.scalar.activation(
                        sq_scr, f_sb[:, kt], Act.Square,
                        accum_out=sal2[:, kt:kt + 1],
                    )
                # saliency + 1/saliency columns for this batch
                nc.scalar.activation(pk_all[:, 0, b, :], sal2, Act.Sqrt)
                nc.vector.reciprocal(pk_all[:, 1, b, :], pk_all[:, 0, b, :])

                # transpose raw f -> fT [128(d), DT, N]
                fT = ftp.tile([128, DT, N], fp32, tag="fT")
                for kt in range(KT):
                    for dt in range(DT):
                        tp = ppB.tile([128, 128], fp32, tag="tps")
                        nc.tensor.transpose(
                            tp, f_sb[:, kt, dt * 128:(dt + 1) * 128], ident
                        )
                        if (kt + dt) % 2 == 0:
                            nc.scalar.copy(
                                fT[:, dt, kt * 128:(kt + 1) * 128], tp
                            )
                        else:
                            nc.vector.tensor_copy(
                                fT[:, dt, kt * 128:(kt + 1) * 128], tp
                            )

                # G_raw = fT.T @ fT, row-scaled by 1/sal on the way out
                for i in range(KT):
                    gps = ppA.tile([128, N], fp32, tag="gps")
                    for h in range(2):
                        for dt in range(DT):
                            nc.tensor.matmul(
                                gps[:, h * 512:(h + 1) * 512],
                                g_ap(fT[:, dt, i * 128:(i + 1) * 128]),
                                g_ap(fT[:, dt, h * 512:(h + 1) * 512]),
                                start=(dt == 0),
                                stop=(dt == DT - 1),
                            )
                    inv_i = pk_all[:, 1, b, i:i + 1]
                    gstage = gsp.tile([128, N], fp32, tag="gstage")
                    nc.vector.tensor_scalar(
                        gstage[:, :512], gps[:, :512], inv_i, None, op0=Alu.mult
                    )
                    nc.scalar.activation(
                        gstage[:, 512:], gps[:, 512:], Act.Copy, scale=inv_i
                    )
                    nc.sync.dma_start(
                        g_wr[b * N + i * 128: b * N + (i + 1) * 128, :], gstage
                    )

            # blocked saliency / inverse-saliency: one transpose each
            salT_ps = ppB.tile([BG, 128], fp32, tag="awt")
            nc.tensor.transpose(
                salT_ps, pk_all[:, 0].rearrange("p a b -> p (a b)"), ident
            )
            nc.scalar.copy(sal_blk, salT_ps)
            invT_ps = ppB.tile([BG, 128], fp32, tag="awt")
            nc.tensor.transpose(
                invT_ps, pk_all[:, 1].rearrange("p a b -> p (a b)"), ident
            )
            nc.vector.tensor_copy(invs_blk, invT_ps)

            # prefetch Phase C feature reloads (independent of g_dr);
            # only bufs-1 pre-barrier or the sync engine deadlocks waiting
            # for buffer releases that happen after the barrier
            N_PREFETCH = 2
            fc_tiles = []
            for b in range(N_PREFETCH):
                f_c = fbp.tile([128, KT, D], fp32, tag="f")
                nc.sync.dma_start(
                    f_c, f_dr[b].rearrange("(kt p) d -> p kt d", p=128)
                )
                fc_tiles.append(f_c)

            # all Gram writes must be visible before indirect gathers
            tc.strict_bb_all_engine_barrier()

            # ---------------- Phase B: 16-step greedy loop ------------------
            nc.vector.memset(mask, 1.0)
            mx8 = mp.tile([BG, 8], fp32)
            ix8 = mp.tile([BG, 8], u32)
            idxf = mp.tile([BG, 1], fp32)
            valT = mp.tile([1, BG], fp32)
            idxT = mp.tile([1, BG], fp32)
            bmax = mp.tile([1, BPC], fp32)
            eq = mp.tile([1, BG], u8)
            code = mp.tile([1, BG], fp32)
            nst = mp.tile([1, BPC], fp32)
            grow = mp.tile([1, BPC], fp32)
            nT = mp.tile([BPC, 1], fp32)
            offs = mp.tile([BG, 1], i32)
            gate = mp.tile([BG, 128], fp32)
            aggw = mp.tile([BG, 128], fp32)
            aggw2 = mp.tile([BG, 128], fp32)
            ctmp = mp.tile([BG, 128], fp32)
            cval = mp.tile([BG, 128], fp32)

            sims = [mp.tile([BG, 128], fp32, name=f"sim{i}") for i in range(2)]
            w1s = [mp.tile([BG, 128], fp32, name=f"w1_{i}") for i in range(2)]

            def emit_deferred(t):
                # off-critical aggregation work for step t
                s = sims[t % 2]
                w = w1s[t % 2]
                nc.vector.tensor_scalar(gate, s, 0.5, None, op0=Alu.is_gt)
                nc.vector.tensor_mul(aggw, w, gate)
                nc.scalar.activation(
                    aggw2, aggw, Act.Copy, accum_out=wsum_p[:, t:t + 1]
                )
                awT_ps = ppB.tile([128, BG], fp32, tag="awt")
                nc.tensor.transpose(awT_ps, aggw, ident[:BG, :BG])
                # awT_ps col j = partition b*8+g ; wT target (kt=g, b)
                nc.scalar.copy(
                    wT[:, :, :, t],
                    awT_ps.rearrange("p (b g) -> p g b", b=BPC),
                )

            for t in range(NS):
                s = sims[t % 2]
                w1 = w1s[t % 2]
                nc.vector.tensor_mul(msal, sal_blk, mask)
                nc.vector.max(out=mx8, in_=msal)
                nc.vector.max_index(out=ix8, in_max=mx8, in_values=msal)
                # local idx -> flat n = g*128 + c
                nc.vector.tensor_scalar(
                    idxf, ix8[:, 0:1], pcol[:, 0:1], None, op0=Alu.add
                )
                valT_ps = ppB.tile([1, BG], fp32, tag="tps")
                nc.tensor.transpose(valT_ps, mx8[:, 0:1], ident[:BG, :BG])
                nc.scalar.copy(valT, valT_ps)
                idxT_ps = ppB.tile([1, BG], fp32, tag="tps")
                nc.tensor.transpose(idxT_ps, idxf, ident[:BG, :BG])
                nc.scalar.copy(idxT, idxT_ps)
                # per-batch max over groups, first-index tiebreak via min-code
                nc.vector.tensor_reduce(
                    bmax, valT.rearrange("o (b g) -> o b g", b=BPC),
                    axis=X, op=Alu.max,
                )
                nc.vector.tensor_tensor(
                    eq.rearrange("o (b g) -> o b g", b=BPC),
                    valT.rearrange("o (b g) -> o b g", b=BPC),
                    bmax.unsqueeze(2).to_broadcast([1, BPC, NG]),
                    op=Alu.is_ge,
                )
                nc.vector.select(code, eq, idxT, crow[0:1, 0:BG])
                nc.vector.tensor_reduce(
                    nst, code.rearrange("o (b g) -> o b g", b=BPC),
                    axis=X, op=Alu.min,
                )
                nc.vector.tensor_tensor(
                    grow, nst, crow[0:1, BG:BG + BPC], op=Alu.add
                )
                nT_ps = ppB.tile([BPC, 1], fp32, tag="tps")
                nc.tensor.transpose(nT_ps, grow, ident[:1, :1])
                nc.scalar.copy(nT, nT_ps)
                rep_ps = ppB.tile([BG, 1], fp32, tag="tps")
                nc.tensor.matmul(rep_ps, E_sb, nT, start=True, stop=True)
                nc.vector.tensor_scalar(
                    offs, rep_ps, 8.0, pcol[:, 1:2], op0=Alu.mult, op1=Alu.add
                )
                nc.gpsimd.indirect_dma_start(
                    out=s,
                    out_offset=None,
                    in_=g_dr,
                    in_offset=IndirectOffsetOnAxis(ap=offs, axis=0),
                )
                if t > 0:
                    emit_deferred(t - 1)
                # critical tail: finish similarity, capture w1, update mask
                nc.vector.tensor_mul(s, s, invs_blk)
                nc.vector.tensor_mul(w1, s, mask)
                nc.vector.tensor_scalar(
                    ctmp, s, 0.0, 1.0, op0=Alu.max, op1=Alu.min
                )
                nc.vector.tensor_scalar(
                    cval, ctmp, -1.0, 1.0, op0=Alu.mult, op1=Alu.add
                )
                nc.vector.tensor_mul(mask, mask, cval)
            emit_deferred(NS - 1)

            # ---------------- Phase C: slot matmuls -------------------------
            # wsum: per-batch totals from per-(b,g) partials
            wsT_ps = ppB.tile([NS, BG], fp32, tag="tps")
            nc.tensor.transpose(wsT_ps, wsum_p, ident[:BG, :BG])
            wsT = mp.tile([NS, BG], fp32)
            nc.scalar.copy(wsT, wsT_ps)
            wsum_b = mp.tile([NS, BPC], fp32)
            nc.vector.tensor_reduce(
                wsum_b, wsT.rearrange("p (b g) -> p b g", b=BPC),
                axis=X, op=Alu.add,
            )
            nc.vector.tensor_scalar(
                wsum_b, wsum_b, 1e-8, None, op0=Alu.add
            )
            recip = mp.tile([NS, BPC], fp32)
            nc.vector.reciprocal(recip, wsum_b)

            for b in range(BPC):
                if b < N_PREFETCH:
                    f_c = fc_tiles[b]
                else:
                    f_c = fbp.tile([128, KT, D], fp32, tag="f")
                    nc.sync.dma_start(
                        f_c, f_dr[b].rearrange("(kt p) d -> p kt d", p=128)
                    )
                sp = ppA.tile([NS, D], fp32, tag="gps")
                for h, (h0, h1) in enumerate([(0, 512), (512, D)]):
                    for kt in range(KT):
                        nc.tensor.matmul(
                            sp[:, h0:h1],
                            s_ap(wT[:, kt, b, :]),
                            s_ap(f_c[:, kt, h0:h1]),
                            start=(kt == 0),
                            stop=(kt == KT - 1),
                        )
                slot_sb = gsp.tile([NS, D], fp32, tag="slot")
                nc.scalar.activation(
                    slot_sb, sp, Act.Copy, scale=recip[:, b:b + 1]
                )
                nc.sync.dma_start(out_dr[b], slot_sb)

    nc.compile()
    return nc


def _get_nc(debug=False):
    key = ("nc", debug, GRAM_DT, SLOT_DT)
    if key not in _CACHE:
        _CACHE[key] = _build_nc(debug)
    return _CACHE[key]


def _consts():
    ident = np.eye(128, dtype=np.float32)
    E = np.zeros((NG, BG), dtype=np.float32)
    for i in range(BG):
        E[i // NG, i] = 1.0
    pcol = np.zeros((BG, 2), dtype=np.float32)
    pcol[:, 0] = (np.arange(BG) % NG) * 128     # g*128
    pcol[:, 1] = np.arange(BG) % NG             # g
    crow = np.zeros((1, 128), dtype=np.float32)
    crow[0, :BG] = 4096.0                       # BIG (> any flat index)
    crow[0, BG:BG + BPC] = np.arange(BPC) * N   # b*1024
    return ident, E, pcol, crow


def _make_in_maps(feats):
    ident, E, pcol, crow = _consts()
    return [
        {
            "features": feats[i * BPC:(i + 1) * BPC],
            "identity": ident,
            "econst": E,
            "pcol": pcol,
            "crow": crow,
        }
        for i in range(NC_CORES)
    ]


def kernel(features, batch_size=None, **_kw):
    from concourse import bass_utils

    nc = _get_nc()
    feats = np.ascontiguousarray(np.asarray(features, dtype=np.float32))
    in_maps = _make_in_maps(feats)
    res = bass_utils.run_bass_kernel_spmd(
        nc, in_maps, core_ids=list(range(NC_CORES))
    )
    outs = [np.asarray(res.results[i]["slots"]) for i in range(NC_CORES)]
    return np.concatenate(outs, axis=0).astype(np.float32)


# revision 23
# speedup vs baseline: 24448.0606x; 10.7942x over previous
import sys

sys.path.insert(0, "/opt/trn_rl_repo")

import numpy as np

# Problem constants (hardcoded per harness contract)
B = 64          # full batch
NC_CORES = 8
BPC = 8         # batches per core
N = 1024
D = 768
NS = 16         # n_slots
KT = 8          # n-tiles of 128
DT = 6          # d-tiles of 128
NG = 8          # column groups per batch in blocked layout (N / 128)
BG = BPC * NG   # 64 partitions of blocked loop state

# Gram matmul dtype: "fp32" (exact, 4 cyc/row) or "fp32r" (1 cyc/row, relaxed)
import os as _os
GRAM_DT = _os.environ.get("KERNEL_GRAM_DT", "fp32")
SLOT_DT = _os.environ.get("KERNEL_SLOT_DT", "fp32")
REPS = int(_os.environ.get("KERNEL_REPS", "1"))
# phase gating for timing attribution: "A", "AB", or "ABC" (full kernel)
PHASES = _os.environ.get("KERNEL_PHASES", "ABC")

_CACHE = {}


def _build_nc(debug=False, reps=None):
    import concourse.bacc as bacc
    import concourse.tile as tile
    import concourse.mybir as mybir
    from concourse.bass import IndirectOffsetOnAxis

    if reps is None:
        reps = REPS

    fp32 = mybir.dt.float32
    fp32r = mybir.dt.float32r
    i32 = mybir.dt.int32
    u32 = mybir.dt.uint32
    u8 = mybir.dt.uint8
    Alu = mybir.AluOpType
    Act = mybir.ActivationFunctionType
    X = mybir.AxisListType.X

    gdt = fp32r if GRAM_DT == "fp32r" else fp32
    sdt = fp32r if SLOT_DT == "fp32r" else fp32

    def g_ap(ap):
        return ap.bitcast(gdt) if gdt != fp32 else ap

    def s_ap(ap):
        return ap.bitcast(sdt) if sdt != fp32 else ap

    nc = bacc.Bacc(
        "TRN2",
        target_bir_lowering=False,
        debug=False,
        enable_asserts=False,
        num_devices=NC_CORES,
    )

    f_dr = nc.dram_tensor("features", [BPC, N, D], fp32, kind="ExternalInput").ap()
    ident_dr = nc.dram_tensor("identity", [128, 128], fp32, kind="ExternalInput").ap()
    ec_dr = nc.dram_tensor("econst", [NG, BG], fp32, kind="ExternalInput").ap()
    pc_dr = nc.dram_tensor("pcol", [BG, 2], fp32, kind="ExternalInput").ap()
    cr_dr = nc.dram_tensor("crow", [1, 128], fp32, kind="ExternalInput").ap()
    out_dr = nc.dram_tensor("slots", [BPC, NS, D], fp32, kind="ExternalOutput").ap()
    # Gram scratch, viewed [BPC*N, N] for writes and [BPC*N*NG, 128] for gathers
    g_dr = nc.dram_tensor("g_scratch", [BPC * N * NG, 128], fp32, kind="Internal").ap()
    g_wr = g_dr.rearrange("(r e) c -> r (e c)", e=NG)  # [BPC*N, N]

    with tile.TileContext(nc) as tc:
        with (
            tc.tile_pool(name="main", bufs=1) as mp,
            tc.tile_pool(name="fbuf", bufs=3) as fbp,
            tc.tile_pool(name="fnt", bufs=2) as ftp,
            tc.tile_pool(name="gst", bufs=4) as gsp,
            tc.tile_pool(name="small", bufs=2) as smp,
            tc.tile_pool(name="psA", bufs=2, space="PSUM") as ppA,
            tc.tile_pool(name="psB", bufs=2, space="PSUM") as ppB,
        ):
            ident = mp.tile([128, 128], fp32)
            nc.sync.dma_start(ident, ident_dr)
            E_sb = mp.tile([NG, BG], fp32)
            nc.sync.dma_start(E_sb, ec_dr)
            pcol = mp.tile([BG, 2], fp32)
            nc.sync.dma_start(pcol, pc_dr)
            # crow: cols 0:64 = BIG (65536)
            crow = mp.tile([1, 128], fp32)
            nc.sync.dma_start(crow, cr_dr)

            for _rep in range(reps):
                _run_once(nc, tc, tile, mybir, IndirectOffsetOnAxis,
                          mp, fbp, ftp, gsp, smp, ppA, ppB,
                          ident, E_sb, pcol, crow,
                          f_dr, out_dr, g_dr, g_wr, g_ap, s_ap)
                if reps > 1:
                    tc.strict_bb_all_engine_barrier()

    nc.compile()
    return nc


def _run_once(nc, tc, tile, mybir, IndirectOffsetOnAxis,
              mp, fbp, ftp, gsp, smp, ppA, ppB,
              ident, E_sb, pcol, crow,
              f_dr, out_dr, g_dr, g_wr, g_ap, s_ap):
    fp32 = mybir.dt.float32
    i32 = mybir.dt.int32
    u32 = mybir.dt.uint32
    u8 = mybir.dt.uint8
    Alu = mybir.AluOpType
    Act = mybir.ActivationFunctionType
    X = mybir.AxisListType.X

    # persistent loop state, blocked layout [64, 128]:
    # partition p = b*8 + g, column c; flat n = g*128 + c
    mask = mp.tile([BG, 128], fp32)
    msal = mp.tile([BG, 128], fp32)
    wT = mp.tile([128, KT, BPC, NS], fp32)   # slot weights, lhsT layout
    wsum_p = mp.tile([BG, NS], fp32)         # per-(b,g) weight sums
    # saliency + 1/saliency staged column-wise, transposed once at end
    pk_all = mp.tile([128, 2, BPC, KT], fp32)

    # ---------------- Phase A: per-batch Gram of normalized features ----
    for b in range(BPC):
        f_sb = fbp.tile([128, KT, D], fp32, tag="f")
        nc.sync.dma_start(
            f_sb, f_dr[b].rearrange("(kt p) d -> p kt d", p=128)
        )
        sal2 = smp.tile([128, KT], fp32, tag="sal2")
        sq_scr = smp.tile([128, D], fp32, tag="sqscr")
        for kt in range(KT):
            nc.scalar.activation(
                sq_scr, f_sb[:, kt], Act.Square,
                accum_out=sal2[:, kt:kt + 1],
            )
        # saliency + 1/saliency columns for this batch
        nc.scalar.activation(pk_all[:, 0, b, :], sal2, Act.Sqrt)
        nc.vector.reciprocal(pk_all[:, 1, b, :], pk_all[:, 0, b, :])
        # normalize rows in place: fn = f / ||f||
        for kt in range(KT):
            nc.vector.tensor_scalar(
                f_sb[:, kt], f_sb[:, kt], pk_all[:, 1, b, kt:kt + 1], None,
                op0=Alu.mult,
            )

        # transpose fn -> fT [128(d), DT, N]
        fT = ftp.tile([128, DT, N], fp32, tag="fT")
        for kt in range(KT):
            for dt in range(DT):
                tp = ppB.tile([128, 128], fp32, tag="tps")
                nc.tensor.transpose(
                    tp, f_sb[:, kt, dt * 128:(dt + 1) * 128], ident
                )
                if (kt + dt) % 2 == 0:
                    nc.scalar.copy(
                        fT[:, dt, kt * 128:(kt + 1) * 128], tp
                    )
                else:
                    nc.vector.tensor_copy(
                        fT[:, dt, kt * 128:(kt + 1) * 128], tp
                    )

        # G = fT.T @ fT (normalized Gram), row tiles -> DRAM
        for i in range(KT):
            gps = ppA.tile([128, N], fp32, tag="gps")
            for h in range(2):
                for dt in range(DT):
                    nc.tensor.matmul(
                        gps[:, h * 512:(h + 1) * 512],
                        g_ap(fT[:, dt, i * 128:(i + 1) * 128]),
                        g_ap(fT[:, dt, h * 512:(h + 1) * 512]),
                        start=(dt == 0),
                        stop=(dt == DT - 1),
                    )
            gstage = gsp.tile([128, N], fp32, tag="gstage")
            nc.vector.tensor_copy(gstage[:, :512], gps[:, :512])
            nc.scalar.copy(gstage[:, 512:], gps[:, 512:])
            nc.sync.dma_start(
                g_wr[b * N + i * 128: b * N + (i + 1) * 128, :], gstage
            )

    # blocked saliency -> msal (initial masked saliency; mask starts at 1)
    salT_ps = ppB.tile([BG, 128], fp32, tag="awt")
    nc.tensor.transpose(
        salT_ps, pk_all[:, 0].rearrange("p a b -> p (a b)"), ident
    )
    nc.scalar.copy(msal, salT_ps)

    # prefetch Phase C feature reloads (independent of g_dr);
    # only bufs-1 pre-barrier or the sync engine deadlocks waiting
    # for buffer releases that happen after the barrier
    N_PREFETCH = 2
    fc_tiles = []
    for b in range(N_PREFETCH):
        f_c = fbp.tile([128, KT, D], fp32, tag="f")
        nc.sync.dma_start(
            f_c, f_dr[b].rearrange("(kt p) d -> p kt d", p=128)
        )
        fc_tiles.append(f_c)

    # all Gram writes must be visible before indirect gathers
    tc.strict_bb_all_engine_barrier()

    if "B" not in PHASES:
        return

    # ---------------- Phase B: 16-step greedy loop ------------------
    nc.vector.memset(mask, 1.0)
    mx8 = mp.tile([BG, 8], fp32)
    ix8 = mp.tile([BG, 8], u32)
    idxf = mp.tile([BG, 1], fp32)
    valT = mp.tile([1, BG], fp32)
    idxT = mp.tile([1, BG], fp32)
    bmax = mp.tile([1, BPC], fp32)
    eq = mp.tile([1, BG], u8)
    code = mp.tile([1, BG], fp32)
    nst = mp.tile([1, BPC], fp32)
    nT = mp.tile([BPC, 1], fp32)
    offs = mp.tile([BG, 1], i32)
    gate = mp.tile([BG, 128], fp32)
    w1 = mp.tile([BG, 128], fp32)
    aggw = mp.tile([BG, 128], fp32)
    aggw2 = mp.tile([BG, 128], fp32)
    um = mp.tile([BG, 128], fp32)
    vtmp = mp.tile([BG, 128], fp32)

    sims = [mp.tile([BG, 128], fp32, name=f"sim{i}") for i in range(2)]
    us = [mp.tile([BG, 128], fp32, name=f"u{i}") for i in range(2)]

    def emit_deferred(t):
        # off-critical aggregation + mask update for step t
        s = sims[t % 2]
        u = us[t % 2]
        nc.vector.tensor_scalar(gate, s, 0.5, None, op0=Alu.is_gt)
        nc.vector.tensor_mul(w1, s, mask)          # sim * mask_t
        nc.vector.tensor_mul(aggw, w1, gate)
        nc.scalar.activation(
            aggw2, aggw, Act.Copy, accum_out=wsum_p[:, t:t + 1]
        )
        awT_ps = ppB.tile([128, BG], fp32, tag="awt")
        nc.tensor.transpose(awT_ps, aggw, ident[:BG, :BG])
        # awT_ps col j = partition b*8+g ; wT target (kt=g, b)
        nc.scalar.copy(
            wT[:, :, :, t],
            awT_ps.rearrange("p (b g) -> p g b", b=BPC),
        )
        # mask = mask * (1 - clip(sim,0,1)) = min(mask*relu(1-sim), mask)
        nc.vector.tensor_mul(um, mask, u)
        nc.vector.tensor_tensor(mask, um, mask, op=Alu.min)

    for t in range(NS):
        s = sims[t % 2]
        u = us[t % 2]
        nc.vector.max(out=mx8, in_=msal)
        nc.vector.max_index(out=ix8, in_max=mx8, in_values=msal)
        # local idx -> global code b*1024 + g*128 + c
        nc.vector.tensor_scalar(
            idxf, ix8[:, 0:1], pcol[:, 0:1], None, op0=Alu.add
        )
        valT_ps = ppB.tile([1, BG], fp32, tag="tps")
        nc.tensor.transpose(valT_ps, mx8[:, 0:1], ident[:BG, :BG])
        nc.scalar.copy(valT, valT_ps)
        idxT_ps = ppB.tile([1, BG], fp32, tag="tps")
        nc.tensor.transpose(idxT_ps, idxf, ident[:BG, :BG])
        nc.scalar.copy(idxT, idxT_ps)
        # per-batch max over groups, first-index tiebreak via min-code
        nc.vector.tensor_reduce(
            bmax, valT.rearrange("o (b g) -> o b g", b=BPC),
            axis=X, op=Alu.max,
        )
        nc.vector.tensor_tensor(
            eq.rearrange("o (b g) -> o b g", b=BPC),
            valT.rearrange("o (b g) -> o b g", b=BPC),
            bmax.unsqueeze(2).to_broadcast([1, BPC, NG]),
            op=Alu.is_ge,
        )
        nc.vector.select(code, eq, idxT, crow[0:1, 0:BG])
        nc.vector.tensor_reduce(
            nst, code.rearrange("o (b g) -> o b g", b=BPC),
            axis=X, op=Alu.min,
        )
        nT_ps = ppB.tile([BPC, 1], fp32, tag="tps")
        nc.tensor.transpose(nT_ps, nst, ident[:1, :1])
        nc.scalar.copy(nT, nT_ps)
        rep_ps = ppB.tile([BG, 1], fp32, tag="tps")
        nc.tensor.matmul(rep_ps, E_sb, nT, start=True, stop=True)
        nc.vector.tensor_scalar(
            offs, rep_ps, 8.0, pcol[:, 1:2], op0=Alu.mult, op1=Alu.add
        )
        nc.gpsimd.indirect_dma_start(
            out=s,
            out_offset=None,
            in_=g_dr,
            in_offset=IndirectOffsetOnAxis(ap=offs, axis=0),
        )
        if t > 0:
            emit_deferred(t - 1)
        # critical tail: msal *= (1 - clip(sim,0,1)), via min trick
        nc.scalar.activation(u, s, Act.Relu, bias=1.0, scale=-1.0)
        nc.vector.tensor_mul(vtmp, msal, u)
        nc.vector.tensor_tensor(msal, vtmp, msal, op=Alu.min)
    emit_deferred(NS - 1)

    if "C" not in PHASES:
        return

    # ---------------- Phase C: slot matmuls -------------------------
    # wsum: per-batch totals from per-(b,g) partials
    wsT_ps = ppB.tile([NS, BG], fp32, tag="tps")
    nc.tensor.transpose(wsT_ps, wsum_p, ident[:BG, :BG])
    wsT = mp.tile([NS, BG], fp32)
    nc.scalar.copy(wsT, wsT_ps)
    wsum_b = mp.tile([NS, BPC], fp32)
    nc.vector.tensor_reduce(
        wsum_b, wsT.rearrange("p (b g) -> p b g", b=BPC),
        axis=X, op=Alu.add,
    )
    nc.vector.tensor_scalar(
        wsum_b, wsum_b, 1e-8, None, op0=Alu.add
    )
    recip = mp.tile([NS, BPC], fp32)
    nc.vector.reciprocal(recip, wsum_b)

    for b in range(BPC):
        if b < len(fc_tiles):
            f_c = fc_tiles[b]
        else:
            f_c = fbp.tile([128, KT, D], fp32, tag="f")
            nc.sync.dma_start(
                f_c, f_dr[b].rearrange("(kt p) d -> p kt d", p=128)
            )
        sp = ppA.tile([NS, D], fp32, tag="gps")
        for h, (h0, h1) in enumerate([(0, 512), (512, D)]):
            for kt in range(KT):
                nc.tensor.matmul(
                    sp[:, h0:h1],
                    s_ap(wT[:, kt, b, :]),
                    s_ap(f_c[:, kt, h0:h1]),
                    start=(kt == 0),
                    stop=(kt == KT - 1),
                )
        slot_sb = gsp.tile([NS, D], fp32, tag="slot")
        nc.scalar.activation(
            slot_sb, sp, Act.Copy, scale=recip[:, b:b + 1]
        )
        nc.sync.dma_start(out_dr[b], slot_sb)


def _get_nc(debug=False, reps=None):
    key = ("nc", debug, GRAM_DT, SLOT_DT, reps if reps is not None else REPS)
    if key not in _CACHE:
        _CACHE[key] = _build_nc(debug, reps=reps)
    return _CACHE[key]


def _consts():
    ident = np.eye(128, dtype=np.float32)
    E = np.zeros((NG, BG), dtype=np.float32)
    for i in range(BG):
        E[i // NG, i] = 1.0
    pcol = np.zeros((BG, 2), dtype=np.float32)
    pcol[:, 0] = (np.arange(BG) // NG) * N + (np.arange(BG) % NG) * 128
    pcol[:, 1] = np.arange(BG) % NG             # g
    crow = np.zeros((1, 128), dtype=np.float32)
    crow[0, :BG] = 65536.0                      # BIG (> any code)
    return ident, E, pcol, crow


def _make_in_maps(feats):
    ident, E, pcol, crow = _consts()
    return [
        {
            "features": feats[i * BPC:(i + 1) * BPC],
            "identity": ident,
            "econst": E,
            "pcol": pcol,
            "crow": crow,
        }
        for i in range(NC_CORES)
    ]


def kernel(features, batch_size=None, **_kw):
    from concourse import bass_utils

    nc = _get_nc(reps=1)
    feats = np.ascontiguousarray(np.asarray(features, dtype=np.float32))
    in_maps = _make_in_maps(feats)
    res = bass_utils.run_bass_kernel_spmd(
        nc, in_maps, core_ids=list(range(NC_CORES))
    )
    outs = [np.asarray(res.results[i]["slots"]) for i in range(NC_CORES)]
    return np.concatenate(outs, axis=0).astype(np.float32)


# revision 25
# speedup vs baseline: 28543.3016x; 1.1675x over previous
import sys

sys.path.insert(0, "/opt/trn_rl_repo")

import numpy as np

# Problem constants (hardcoded per harness contract)
B = 64          # full batch
NC_CORES = 8
BPC = 8         # batches per core
N = 1024
D = 768
NS = 16         # n_slots
KT = 8          # n-tiles of 128
DT = 6          # d-tiles of 128
NG = 8          # column groups per batch in blocked layout (N / 128)
BG = BPC * NG   # 64 partitions of blocked loop state

# Gram matmul dtype: "fp32" (exact, 4 cyc/row) or "fp32r" (1 cyc/row, relaxed)
import os as _os
GRAM_DT = _os.environ.get("KERNEL_GRAM_DT", "fp32")
SLOT_DT = _os.environ.get("KERNEL_SLOT_DT", "fp32")
REPS = int(_os.environ.get("KERNEL_REPS", "1"))
# phase gating for timing attribution: "A", "AB", or "ABC" (full kernel)
PHASES = _os.environ.get("KERNEL_PHASES", "ABC")

_CACHE = {}


def _build_nc(debug=False, reps=None):
    import concourse.bacc as bacc
    import concourse.tile as tile
    import concourse.mybir as mybir
    from concourse.bass import IndirectOffsetOnAxis

    if reps is None:
        reps = REPS

    fp32 = mybir.dt.float32
    fp32r = mybir.dt.float32r
    i32 = mybir.dt.int32
    u32 = mybir.dt.uint32
    u8 = mybir.dt.uint8
    Alu = mybir.AluOpType
    Act = mybir.ActivationFunctionType
    X = mybir.AxisListType.X

    gdt = fp32r if GRAM_DT == "fp32r" else fp32
    sdt = fp32r if SLOT_DT == "fp32r" else fp32

    def g_ap(ap):
        return ap.bitcast(gdt) if gdt != fp32 else ap

    def s_ap(ap):
        return ap.bitcast(sdt) if sdt != fp32 else ap

    nc = bacc.Bacc(
        "TRN2",
        target_bir_lowering=False,
        debug=False,
        enable_asserts=False,
        num_devices=NC_CORES,
    )

    f_dr = nc.dram_tensor("features", [BPC, N, D], fp32, kind="ExternalInput").ap()
    ident_dr = nc.dram_tensor("identity", [128, 128], fp32, kind="ExternalInput").ap()
    ec_dr = nc.dram_tensor("econst", [NG, BG], fp32, kind="ExternalInput").ap()
    pc_dr = nc.dram_tensor("pcol", [BG, 2], fp32, kind="ExternalInput").ap()
    cr_dr = nc.dram_tensor("crow", [1, 128], fp32, kind="ExternalInput").ap()
    out_dr = nc.dram_tensor("slots", [BPC, NS, D], fp32, kind="ExternalOutput").ap()
    # Gram scratch, viewed [BPC*N, N] for writes and [BPC*N*NG, 128] for gathers
    g_dr = nc.dram_tensor("g_scratch", [BPC * N * NG, 128], fp32, kind="Internal").ap()
    g_wr = g_dr.rearrange("(r e) c -> r (e c)", e=NG)  # [BPC*N, N]

    with tile.TileContext(nc) as tc:
        with (
            tc.tile_pool(name="main", bufs=1) as mp,
            tc.tile_pool(name="fbuf", bufs=3) as fbp,
            tc.tile_pool(name="fnt", bufs=2) as ftp,
            tc.tile_pool(name="gst", bufs=4) as gsp,
            tc.tile_pool(name="small", bufs=2) as smp,
            tc.tile_pool(name="psA", bufs=2, space="PSUM") as ppA,
            tc.tile_pool(name="psB", bufs=2, space="PSUM") as ppB,
        ):
            ident = mp.tile([128, 128], fp32)
            nc.sync.dma_start(ident, ident_dr)
            E_sb = mp.tile([NG, BG], fp32)
            nc.sync.dma_start(E_sb, ec_dr)
            pcol = mp.tile([BG, 2], fp32)
            nc.sync.dma_start(pcol, pc_dr)
            # crow: cols 0:64 = BIG (65536)
            crow = mp.tile([1, 128], fp32)
            nc.sync.dma_start(crow, cr_dr)

            for _rep in range(reps):
                _run_once(nc, tc, tile, mybir, IndirectOffsetOnAxis,
                          mp, fbp, ftp, gsp, smp, ppA, ppB,
                          ident, E_sb, pcol, crow,
                          f_dr, out_dr, g_dr, g_wr, g_ap, s_ap)
                if reps > 1:
                    tc.strict_bb_all_engine_barrier()

    nc.compile()
    return nc


def _run_once(nc, tc, tile, mybir, IndirectOffsetOnAxis,
              mp, fbp, ftp, gsp, smp, ppA, ppB,
              ident, E_sb, pcol, crow,
              f_dr, out_dr, g_dr, g_wr, g_ap, s_ap):
    fp32 = mybir.dt.float32
    i32 = mybir.dt.int32
    u32 = mybir.dt.uint32
    u8 = mybir.dt.uint8
    Alu = mybir.AluOpType
    Act = mybir.ActivationFunctionType
    X = mybir.AxisListType.X

    # persistent loop state, blocked layout [64, 128]:
    # partition p = b*8 + g, column c; flat n = g*128 + c
    mask = mp.tile([BG, 128], fp32)
    msal = mp.tile([BG, 128], fp32)
    wT = mp.tile([128, KT, BPC, NS], fp32)   # slot weights, lhsT layout
    wsum_p = mp.tile([BG, NS], fp32)         # per-(b,g) weight sums
    # saliency + 1/saliency staged column-wise, transposed once at end
    pk_all = mp.tile([128, 2, BPC, KT], fp32)

    # ---------------- Phase A: per-batch Gram of normalized features ----
    for b in range(BPC):
        f_sb = fbp.tile([128, KT, D], fp32, tag="f")
        nc.sync.dma_start(
            f_sb, f_dr[b].rearrange("(kt p) d -> p kt d", p=128)
        )
        sal2 = smp.tile([128, KT], fp32, tag="sal2")
        sq_scr = smp.tile([128, D], fp32, tag="sqscr")
        for kt in range(KT):
            nc.scalar.activation(
                sq_scr, f_sb[:, kt], Act.Square,
                accum_out=sal2[:, kt:kt + 1],
            )
        # saliency + 1/saliency columns for this batch
        nc.scalar.activation(pk_all[:, 0, b, :], sal2, Act.Sqrt)
        nc.vector.reciprocal(pk_all[:, 1, b, :], pk_all[:, 0, b, :])
        # normalize rows in place: fn = f / ||f||
        for kt in range(KT):
            nc.vector.tensor_scalar(
                f_sb[:, kt], f_sb[:, kt], pk_all[:, 1, b, kt:kt + 1], None,
                op0=Alu.mult,
            )

        # transpose fn -> fT [128(d), DT, N]
        fT = ftp.tile([128, DT, N], fp32, tag="fT")
        for kt in range(KT):
            for dt in range(DT):
                tp = ppB.tile([128, 128], fp32, tag="tps")
                nc.tensor.transpose(
                    tp, f_sb[:, kt, dt * 128:(dt + 1) * 128], ident
                )
                if (kt + dt) % 2 == 0:
                    nc.scalar.copy(
                        fT[:, dt, kt * 128:(kt + 1) * 128], tp
                    )
                else:
                    nc.vector.tensor_copy(
                        fT[:, dt, kt * 128:(kt + 1) * 128], tp
                    )

        # G = fT.T @ fT (normalized Gram), exploiting symmetry: compute
        # only upper-triangle 256-wide chunks; mirror blocks below the
        # diagonal via PE transpose (bit-identical: same products, same
        # PE reduction order over d, same PSUM chunk order)
        for i in range(KT):
            c0 = (i // 2) * 256
            gps = ppA.tile([128, N], fp32, tag="gps")
            for jc in range(i // 2, 4):
                for dt in range(DT):
                    nc.tensor.matmul(
                        gps[:, jc * 256:(jc + 1) * 256],
                        g_ap(fT[:, dt, i * 128:(i + 1) * 128]),
                        g_ap(fT[:, dt, jc * 256:jc * 256 + 256]),
                        start=(dt == 0),
                        stop=(dt == DT - 1),
                    )
            w = N - c0
            mid = c0 + w // 2  # split active region across two engines
            gstage = gsp.tile([128, N], fp32, tag="gstage")
            nc.vector.tensor_copy(gstage[:, c0:mid], gps[:, c0:mid])
            nc.scalar.copy(gstage[:, mid:], gps[:, mid:])
            nc.sync.dma_start(
                g_wr[b * N + i * 128: b * N + (i + 1) * 128, c0:],
                gstage[:, c0:],
            )
            # mirror blocks (j, i) for j > i, except those row j computes
            # directly (j == i+1 with j odd shares row j's first chunk)
            for j in range(i + 1, KT):
                if j == i + 1 and j % 2 == 1:
                    continue
                raw_sb = gsp.tile([128, 128], fp32, tag="mir")
                if j % 2 == 0:
                    nc.vector.tensor_copy(raw_sb, gps[:, j * 128:(j + 1) * 128])
                else:
                    nc.scalar.copy(raw_sb, gps[:, j * 128:(j + 1) * 128])
                tps = ppB.tile([128, 128], fp32, tag="tps")
                nc.tensor.transpose(tps, raw_sb, ident)
                mstage = gsp.tile([128, 128], fp32, tag="mir")
                if j % 2 == 0:
                    nc.scalar.copy(mstage, tps)
                else:
                    nc.vector.tensor_copy(mstage, tps)
                nc.sync.dma_start(
                    g_wr[b * N + j * 128: b * N + (j + 1) * 128,
                         i * 128:(i + 1) * 128],
                    mstage,
                )

    # blocked saliency -> msal (initial masked saliency; mask starts at 1)
    salT_ps = ppB.tile([BG, 128], fp32, tag="awt")
    nc.tensor.transpose(
        salT_ps, pk_all[:, 0].rearrange("p a b -> p (a b)"), ident
    )
    nc.scalar.copy(msal, salT_ps)

    # prefetch Phase C feature reloads (independent of g_dr);
    # only bufs-1 pre-barrier or the sync engine deadlocks waiting
    # for buffer releases that happen after the barrier
    N_PREFETCH = 2
    fc_tiles = []
    for b in range(N_PREFETCH):
        f_c = fbp.tile([128, KT, D], fp32, tag="f")
        nc.sync.dma_start(
            f_c, f_dr[b].rearrange("(kt p) d -> p kt d", p=128)
        )
        fc_tiles.append(f_c)

    # all Gram writes must be visible before indirect gathers
    tc.strict_bb_all_engine_barrier()

    if "B" not in PHASES:
        return

    # ---------------- Phase B: 16-step greedy loop ------------------
    nc.vector.memset(mask, 1.0)
    mx8 = mp.tile([BG, 8], fp32)
    ix8 = mp.tile([BG, 8], u32)
    idxf = mp.tile([BG, 1], fp32)
    valT = mp.tile([1, BG], fp32)
    idxT = mp.tile([1, BG], fp32)
    bmax = mp.tile([1, BPC], fp32)
    eq = mp.tile([1, BG], u8)
    code = mp.tile([1, BG], fp32)
    nst = mp.tile([1, BPC], fp32)
    nT = mp.tile([BPC, 1], fp32)
    offs = mp.tile([BG, 1], i32)
    gate = mp.tile([BG, 128], fp32)
    w1 = mp.tile([BG, 128], fp32)
    aggw = mp.tile([BG, 128], fp32)
    aggw2 = mp.tile([BG, 128], fp32)
    um = mp.tile([BG, 128], fp32)
    vtmp = mp.tile([BG, 128], fp32)

    sims = [mp.tile([BG, 128], fp32, name=f"sim{i}") for i in range(2)]
    us = [mp.tile([BG, 128], fp32, name=f"u{i}") for i in range(2)]

    def emit_deferred(t):
        # off-critical aggregation + mask update for step t
        s = sims[t % 2]
        u = us[t % 2]
        nc.vector.tensor_scalar(gate, s, 0.5, None, op0=Alu.is_gt)
        nc.vector.tensor_mul(w1, s, mask)          # sim * mask_t
        nc.vector.tensor_mul(aggw, w1, gate)
        nc.scalar.activation(
            aggw2, aggw, Act.Copy, accum_out=wsum_p[:, t:t + 1]
        )
        awT_ps = ppB.tile([128, BG], fp32, tag="awt")
        nc.tensor.transpose(awT_ps, aggw, ident[:BG, :BG])
        # awT_ps col j = partition b*8+g ; wT target (kt=g, b)
        nc.scalar.copy(
            wT[:, :, :, t],
            awT_ps.rearrange("p (b g) -> p g b", b=BPC),
        )
        # mask = mask * (1 - clip(sim,0,1)) = min(mask*relu(1-sim), mask)
        nc.vector.tensor_mul(um, mask, u)
        nc.vector.tensor_tensor(mask, um, mask, op=Alu.min)

    for t in range(NS):
        s = sims[t % 2]
        u = us[t % 2]
        nc.vector.max(out=mx8, in_=msal)
        nc.vector.max_index(out=ix8, in_max=mx8, in_values=msal)
        # local idx -> global code b*1024 + g*128 + c
        nc.vector.tensor_scalar(
            idxf, ix8[:, 0:1], pcol[:, 0:1], None, op0=Alu.add
        )
        valT_ps = ppB.tile([1, BG], fp32, tag="tps")
        nc.tensor.transpose(valT_ps, mx8[:, 0:1], ident[:BG, :BG])
        nc.scalar.copy(valT, valT_ps)
        idxT_ps = ppB.tile([1, BG], fp32, tag="tps")
        nc.tensor.transpose(idxT_ps, idxf, ident[:BG, :BG])
        nc.scalar.copy(idxT, idxT_ps)
        # per-batch max over groups, first-index tiebreak via min-code
        nc.vector.tensor_reduce(
            bmax, valT.rearrange("o (b g) -> o b g", b=BPC),
            axis=X, op=Alu.max,
        )
        nc.vector.tensor_tensor(
            eq.rearrange("o (b g) -> o b g", b=BPC),
            valT.rearrange("o (b g) -> o b g", b=BPC),
            bmax.unsqueeze(2).to_broadcast([1, BPC, NG]),
            op=Alu.is_ge,
        )
        nc.vector.select(code, eq, idxT, crow[0:1, 0:BG])
        nc.vector.tensor_reduce(
            nst, code.rearrange("o (b g) -> o b g", b=BPC),
            axis=X, op=Alu.min,
        )
        nT_ps = ppB.tile([BPC, 1], fp32, tag="tps")
        nc.tensor.transpose(nT_ps, nst, ident[:1, :1])
        nc.scalar.copy(nT, nT_ps)
        rep_ps = ppB.tile([BG, 1], fp32, tag="tps")
        nc.tensor.matmul(rep_ps, E_sb, nT, start=True, stop=True)
        nc.vector.tensor_scalar(
            offs, rep_ps, 8.0, pcol[:, 1:2], op0=Alu.mult, op1=Alu.add
        )
        nc.gpsimd.indirect_dma_start(
            out=s,
            out_offset=None,
            in_=g_dr,
            in_offset=IndirectOffsetOnAxis(ap=offs, axis=0),
        )
        if t > 0:
            emit_deferred(t - 1)
        # critical tail: msal *= (1 - clip(sim,0,1)), via min trick
        nc.scalar.activation(u, s, Act.Relu, bias=1.0, scale=-1.0)
        nc.vector.tensor_mul(vtmp, msal, u)
        nc.vector.tensor_tensor(msal, vtmp, msal, op=Alu.min)
    emit_deferred(NS - 1)

    if "C" not in PHASES:
        return

    # ---------------- Phase C: slot matmuls -------------------------
    # wsum: per-batch totals from per-(b,g) partials
    wsT_ps = ppB.tile([NS, BG], fp32, tag="tps")
    nc.tensor.transpose(wsT_ps, wsum_p, ident[:BG, :BG])
    wsT = mp.tile([NS, BG], fp32)
    nc.scalar.copy(wsT, wsT_ps)
    wsum_b = mp.tile([NS, BPC], fp32)
    nc.vector.tensor_reduce(
        wsum_b, wsT.rearrange("p (b g) -> p b g", b=BPC),
        axis=X, op=Alu.add,
    )
    nc.vector.tensor_scalar(
        wsum_b, wsum_b, 1e-8, None, op0=Alu.add
    )
    recip = mp.tile([NS, BPC], fp32)
    nc.vector.reciprocal(recip, wsum_b)

    for b in range(BPC):
        if b < len(fc_tiles):
            f_c = fc_tiles[b]
        else:
            f_c = fbp.tile([128, KT, D], fp32, tag="f")
            nc.sync.dma_start(
                f_c, f_dr[b].rearrange("(kt p) d -> p kt d", p=128)
            )
        sp = ppA.tile([NS, D], fp32, tag="gps")
        for h, (h0, h1) in enumerate([(0, 512), (512, D)]):
            for kt in range(KT):
                nc.tensor.matmul(
                    sp[:, h0:h1],
                    s_ap(wT[:, kt, b, :]),
                    s_ap(f_c[:, kt, h0:h1]),
                    start=(kt == 0),
                    stop=(kt == KT - 1),
                )
        slot_sb = gsp.tile([NS, D], fp32, tag="slot")
        nc.scalar.activation(
            slot_sb, sp, Act.Copy, scale=recip[:, b:b + 1]
        )
        nc.sync.dma_start(out_dr[b], slot_sb)


def _get_nc(debug=False, reps=None):
    key = ("nc", debug, GRAM_DT, SLOT_DT, reps if reps is not None else REPS)
    if key not in _CACHE:
        _CACHE[key] = _build_nc(debug, reps=reps)
    return _CACHE[key]


def _consts():
    ident = np.eye(128, dtype=np.float32)
    E = np.zeros((NG, BG), dtype=np.float32)
    for i in range(BG):
        E[i // NG, i] = 1.0
    pcol = np.zeros((BG, 2), dtype=np.float32)
    pcol[:, 0] = (np.arange(BG) // NG) * N + (np.arange(BG) % NG) * 128
    pcol[:, 1] = np.arange(BG) % NG             # g
    crow = np.zeros((1, 128), dtype=np.float32)
    crow[0, :BG] = 65536.0                      # BIG (> any code)
    return ident, E, pcol, crow


def _make_in_maps(feats):
    ident, E, pcol, crow = _consts()
    return [
        {
            "features": feats[i * BPC:(i + 1) * BPC],
            "identity": ident,
            "econst": E,
            "pcol": pcol,
            "crow": crow,
        }
        for i in range(NC_CORES)
    ]


def kernel(features, batch_size=None, **_kw):
    from concourse import bass_utils

    nc = _get_nc(reps=1)
    feats = np.ascontiguousarray(np.asarray(features, dtype=np.float32))
    in_maps = _make_in_maps(feats)
    res = bass_utils.run_bass_kernel_spmd(
        nc, in_maps, core_ids=list(range(NC_CORES))
    )
    outs = [np.asarray(res.results[i]["slots"]) for i in range(NC_CORES)]
    return np.concatenate(outs, axis=0).astype(np.float32)


# revision 28
# speedup vs baseline: 125489.3827x; 4.3965x over previous
import sys

sys.path.insert(0, "/opt/trn_rl_repo")

import numpy as np

# Problem constants (hardcoded per harness contract)
B = 64          # full batch
NC_CORES = 8
BPC = 8         # batches per core
N = 1024
D = 768
NS = 16         # n_slots
KT = 8          # n-tiles of 128
DT = 6          # d-tiles of 128
NG = 8          # column groups per batch in blocked layout (N / 128)
BG = BPC * NG   # 64 partitions of blocked loop state

# Gram matmul dtype: "fp32" (exact, 4 cyc/row) or "fp32r" (1 cyc/row, relaxed)
import os as _os
GRAM_DT = _os.environ.get("KERNEL_GRAM_DT", "fp32")
SLOT_DT = _os.environ.get("KERNEL_SLOT_DT", "fp32")
REPS = int(_os.environ.get("KERNEL_REPS", "1"))
# phase gating for timing attribution: "A", "AB", or "ABC" (full kernel)
PHASES = _os.environ.get("KERNEL_PHASES", "ABC")

_CACHE = {}


def _build_nc(debug=False, reps=None):
    import concourse.bacc as bacc
    import concourse.tile as tile
    import concourse.mybir as mybir
    from concourse.bass import IndirectOffsetOnAxis

    if reps is None:
        reps = REPS

    fp32 = mybir.dt.float32
    fp32r = mybir.dt.float32r
    i32 = mybir.dt.int32
    u32 = mybir.dt.uint32
    u8 = mybir.dt.uint8
    Alu = mybir.AluOpType
    Act = mybir.ActivationFunctionType
    X = mybir.AxisListType.X

    gdt = fp32r if GRAM_DT == "fp32r" else fp32
    sdt = fp32r if SLOT_DT == "fp32r" else fp32

    def g_ap(ap):
        return ap.bitcast(gdt) if gdt != fp32 else ap

    def s_ap(ap):
        return ap.bitcast(sdt) if sdt != fp32 else ap

    nc = bacc.Bacc(
        "TRN2",
        target_bir_lowering=False,
        debug=False,
        enable_asserts=False,
        num_devices=NC_CORES,
    )

    f_dr = nc.dram_tensor("features", [BPC, N, D], fp32, kind="ExternalInput").ap()
    ident_dr = nc.dram_tensor("identity", [128, 128], fp32, kind="ExternalInput").ap()
    ec_dr = nc.dram_tensor("econst", [NG, BG], fp32, kind="ExternalInput").ap()
    pc_dr = nc.dram_tensor("pcol", [BG, 2], fp32, kind="ExternalInput").ap()
    cr_dr = nc.dram_tensor("crow", [1, 128], fp32, kind="ExternalInput").ap()
    out_dr = nc.dram_tensor("slots", [BPC, NS, D], fp32, kind="ExternalOutput").ap()
    # Gram scratch, viewed [BPC*N, N] for writes and [BPC*N*NG, 128] for gathers
    g_dr = nc.dram_tensor("g_scratch", [BPC * N * NG, 128], fp32, kind="Internal").ap()
    g_wr = g_dr.rearrange("(r e) c -> r (e c)", e=NG)  # [BPC*N, N]

    with tile.TileContext(nc) as tc:
        with (
            tc.tile_pool(name="main", bufs=1) as mp,
            tc.tile_pool(name="fbuf", bufs=3) as fbp,
            tc.tile_pool(name="fnt", bufs=2) as ftp,
            tc.tile_pool(name="gst", bufs=4) as gsp,
            tc.tile_pool(name="small", bufs=2) as smp,
            tc.tile_pool(name="psA", bufs=2, space="PSUM") as ppA,
            tc.tile_pool(name="psB", bufs=2, space="PSUM") as ppB,
        ):
            ident = mp.tile([128, 128], fp32)
            nc.sync.dma_start(ident, ident_dr)
            E_sb = mp.tile([NG, BG], fp32)
            nc.sync.dma_start(E_sb, ec_dr)
            pcol = mp.tile([BG, 2], fp32)
            nc.sync.dma_start(pcol, pc_dr)
            # crow: cols 0:64 = BIG (65536)
            crow = mp.tile([1, 128], fp32)
            nc.sync.dma_start(crow, cr_dr)

            for _rep in range(reps):
                _run_once(nc, tc, tile, mybir, IndirectOffsetOnAxis,
                          mp, fbp, ftp, gsp, smp, ppA, ppB,
                          ident, E_sb, pcol, crow,
                          f_dr, out_dr, g_dr, g_wr, g_ap, s_ap)
                if reps > 1:
                    tc.strict_bb_all_engine_barrier()

    nc.compile()
    return nc


def _run_once(nc, tc, tile, mybir, IndirectOffsetOnAxis,
              mp, fbp, ftp, gsp, smp, ppA, ppB,
              ident, E_sb, pcol, crow,
              f_dr, out_dr, g_dr, g_wr, g_ap, s_ap):
    fp32 = mybir.dt.float32
    i32 = mybir.dt.int32
    u32 = mybir.dt.uint32
    u8 = mybir.dt.uint8
    Alu = mybir.AluOpType
    Act = mybir.ActivationFunctionType
    X = mybir.AxisListType.X

    # persistent loop state, blocked layout [64, 128]:
    # partition p = b*8 + g, column c; flat n = g*128 + c
    mask = mp.tile([BG, 128], fp32)
    msal = mp.tile([BG, 128], fp32)
    wT = mp.tile([128, KT, BPC, NS], fp32)   # slot weights, lhsT layout
    wsum_p = mp.tile([BG, NS], fp32)         # per-(b,g) weight sums
    # saliency + 1/saliency staged column-wise, transposed once at end
    pk_all = mp.tile([128, 2, BPC, KT], fp32)

    # ---------------- Phase A: per-batch Gram of normalized features ----
    for b in range(BPC):
        f_sb = fbp.tile([128, KT, D], fp32, tag="f")
        nc.sync.dma_start(
            f_sb, f_dr[b].rearrange("(kt p) d -> p kt d", p=128)
        )
        sal2 = smp.tile([128, KT], fp32, tag="sal2")
        sq_scr = smp.tile([128, D], fp32, tag="sqscr")
        for kt in range(KT):
            nc.scalar.activation(
                sq_scr, f_sb[:, kt], Act.Square,
                accum_out=sal2[:, kt:kt + 1],
            )
        # saliency + 1/saliency columns for this batch
        nc.scalar.activation(pk_all[:, 0, b, :], sal2, Act.Sqrt)
        nc.vector.reciprocal(pk_all[:, 1, b, :], pk_all[:, 0, b, :])
        # normalize rows in place: fn = f / ||f||
        for kt in range(KT):
            nc.vector.tensor_scalar(
                f_sb[:, kt], f_sb[:, kt], pk_all[:, 1, b, kt:kt + 1], None,
                op0=Alu.mult,
            )

        # transpose fn -> fT [128(d), DT, N]
        fT = ftp.tile([128, DT, N], fp32, tag="fT")
        for kt in range(KT):
            for dt in range(DT):
                tp = ppB.tile([128, 128], fp32, tag="tps")
                nc.tensor.transpose(
                    tp, f_sb[:, kt, dt * 128:(dt + 1) * 128], ident
                )
                if (kt + dt) % 2 == 0:
                    nc.scalar.copy(
                        fT[:, dt, kt * 128:(kt + 1) * 128], tp
                    )
                else:
                    nc.vector.tensor_copy(
                        fT[:, dt, kt * 128:(kt + 1) * 128], tp
                    )

        # G = fT.T @ fT (normalized Gram), exploiting symmetry: compute
        # only upper-triangle 256-wide chunks; mirror blocks below the
        # diagonal via PE transpose (bit-identical: same products, same
        # PE reduction order over d, same PSUM chunk order)
        for i in range(KT):
            c0 = (i // 2) * 256
            gps = ppA.tile([128, N], fp32, tag="gps")
            for jc in range(i // 2, 4):
                for dt in range(DT):
                    nc.tensor.matmul(
                        gps[:, jc * 256:(jc + 1) * 256],
                        g_ap(fT[:, dt, i * 128:(i + 1) * 128]),
                        g_ap(fT[:, dt, jc * 256:jc * 256 + 256]),
                        start=(dt == 0),
                        stop=(dt == DT - 1),
                    )
            w = N - c0
            mid = c0 + w // 2  # split active region across two engines
            gstage = gsp.tile([128, N], fp32, tag="gstage")
            nc.vector.tensor_copy(gstage[:, c0:mid], gps[:, c0:mid])
            nc.scalar.copy(gstage[:, mid:], gps[:, mid:])
            nc.sync.dma_start(
                g_wr[b * N + i * 128: b * N + (i + 1) * 128, c0:],
                gstage[:, c0:],
            )
            # mirror blocks (j, i) for j > i, except those row j computes
            # directly (j == i+1 with j odd shares row j's first chunk)
            for j in range(i + 1, KT):
                if j == i + 1 and j % 2 == 1:
                    continue
                raw_sb = gsp.tile([128, 128], fp32, tag="mir")
                if j % 2 == 0:
                    nc.vector.tensor_copy(raw_sb, gps[:, j * 128:(j + 1) * 128])
                else:
                    nc.scalar.copy(raw_sb, gps[:, j * 128:(j + 1) * 128])
                tps = ppB.tile([128, 128], fp32, tag="tps")
                nc.tensor.transpose(tps, raw_sb, ident)
                mstage = gsp.tile([128, 128], fp32, tag="mir")
                if j % 2 == 0:
                    nc.scalar.copy(mstage, tps)
                else:
                    nc.vector.tensor_copy(mstage, tps)
                nc.sync.dma_start(
                    g_wr[b * N + j * 128: b * N + (j + 1) * 128,
                         i * 128:(i + 1) * 128],
                    mstage,
                )

    # blocked saliency -> msal (initial masked saliency; mask starts at 1)
    salT_ps = ppB.tile([BG, 128], fp32, tag="awt")
    nc.tensor.transpose(
        salT_ps, pk_all[:, 0].rearrange("p a b -> p (a b)"), ident
    )
    nc.scalar.copy(msal, salT_ps)

    # prefetch Phase C feature reloads (independent of g_dr);
    # only bufs-1 pre-barrier or the sync engine deadlocks waiting
    # for buffer releases that happen after the barrier
    N_PREFETCH = 2
    fc_tiles = []
    for b in range(N_PREFETCH):
        f_c = fbp.tile([128, KT, D], fp32, tag="f")
        nc.sync.dma_start(
            f_c, f_dr[b].rearrange("(kt p) d -> p kt d", p=128)
        )
        fc_tiles.append(f_c)

    # all Gram writes must be visible before indirect gathers
    tc.strict_bb_all_engine_barrier()

    if "B" not in PHASES:
        return

    # ---------------- Phase B: 16-step greedy loop ------------------
    nc.vector.memset(mask, 1.0)
    mx8 = mp.tile([BG, 8], fp32)
    ix8 = mp.tile([BG, 8], u32)
    idxf = mp.tile([BG, 1], fp32)
    valT = mp.tile([1, BG], fp32)
    idxT = mp.tile([1, BG], fp32)
    bmax = mp.tile([1, BPC], fp32)
    eq = mp.tile([1, BG], u8)
    code = mp.tile([1, BG], fp32)
    nst = mp.tile([1, BPC], fp32)
    nT = mp.tile([BPC, 1], fp32)
    offs = mp.tile([BG, 1], i32)
    gate = mp.tile([BG, 128], fp32)
    w1 = mp.tile([BG, 128], fp32)
    aggw = mp.tile([BG, 128], fp32)
    aggw2 = mp.tile([BG, 128], fp32)
    um = mp.tile([BG, 128], fp32)
    vtmp = mp.tile([BG, 128], fp32)

    sims = [mp.tile([BG, 128], fp32, name=f"sim{i}") for i in range(2)]
    us = [mp.tile([BG, 128], fp32, name=f"u{i}") for i in range(2)]

    def emit_deferred(t):
        # off-critical aggregation + mask update for step t
        s = sims[t % 2]
        u = us[t % 2]
        nc.vector.tensor_scalar(gate, s, 0.5, None, op0=Alu.is_gt)
        nc.vector.tensor_mul(w1, s, mask)          # sim * mask_t
        nc.vector.tensor_mul(aggw, w1, gate)
        nc.scalar.activation(
            aggw2, aggw, Act.Copy, accum_out=wsum_p[:, t:t + 1]
        )
        awT_ps = ppB.tile([128, BG], fp32, tag="awt")
        nc.tensor.transpose(awT_ps, aggw, ident[:BG, :BG])
        # awT_ps col j = partition b*8+g ; wT target (kt=g, b)
        nc.scalar.copy(
            wT[:, :, :, t],
            awT_ps.rearrange("p (b g) -> p g b", b=BPC),
        )
        # mask = mask * (1 - clip(sim,0,1)) = min(mask*relu(1-sim), mask)
        nc.vector.tensor_mul(um, mask, u)
        nc.vector.tensor_tensor(mask, um, mask, op=Alu.min)

    for t in range(NS):
        s = sims[t % 2]
        u = us[t % 2]
        nc.vector.max(out=mx8, in_=msal)
        nc.vector.max_index(out=ix8, in_max=mx8, in_values=msal)
        # local idx -> global code b*1024 + g*128 + c
        nc.vector.tensor_scalar(
            idxf, ix8[:, 0:1], pcol[:, 0:1], None, op0=Alu.add
        )
        valT_ps = ppB.tile([1, BG], fp32, tag="tps")
        nc.tensor.transpose(valT_ps, mx8[:, 0:1], ident[:BG, :BG])
        nc.scalar.copy(valT, valT_ps)
        idxT_ps = ppB.tile([1, BG], fp32, tag="tps")
        nc.tensor.transpose(idxT_ps, idxf, ident[:BG, :BG])
        nc.scalar.copy(idxT, idxT_ps)
        # per-batch max over groups, first-index tiebreak via min-code
        nc.vector.tensor_reduce(
            bmax, valT.rearrange("o (b g) -> o b g", b=BPC),
            axis=X, op=Alu.max,
        )
        nc.vector.tensor_tensor(
            eq.rearrange("o (b g) -> o b g", b=BPC),
            valT.rearrange("o (b g) -> o b g", b=BPC),
            bmax.unsqueeze(2).to_broadcast([1, BPC, NG]),
            op=Alu.is_ge,
        )
        nc.vector.select(code, eq, idxT, crow[0:1, 0:BG])
        nc.vector.tensor_reduce(
            nst, code.rearrange("o (b g) -> o b g", b=BPC),
            axis=X, op=Alu.min,
        )
        nT_ps = ppB.tile([BPC, 1], fp32, tag="tps")
        nc.tensor.transpose(nT_ps, nst, ident[:1, :1])
        nc.scalar.copy(nT, nT_ps)
        rep_ps = ppB.tile([BG, 1], fp32, tag="tps")
        nc.tensor.matmul(rep_ps, E_sb, nT, start=True, stop=True)
        nc.vector.tensor_scalar(
            offs, rep_ps, 8.0, pcol[:, 1:2], op0=Alu.mult, op1=Alu.add
        )
        nc.gpsimd.indirect_dma_start(
            out=s,
            out_offset=None,
            in_=g_dr,
            in_offset=IndirectOffsetOnAxis(ap=offs, axis=0),
        )
        if t > 0:
            emit_deferred(t - 1)
        # critical tail: msal *= (1 - clip(sim,0,1)), via min trick
        nc.scalar.activation(u, s, Act.Relu, bias=1.0, scale=-1.0)
        nc.vector.tensor_mul(vtmp, msal, u)
        nc.vector.tensor_tensor(msal, vtmp, msal, op=Alu.min)
    emit_deferred(NS - 1)

    if "C" not in PHASES:
        return

    # ---------------- Phase C: slot matmuls -------------------------
    # wsum: per-batch totals from per-(b,g) partials
    wsT_ps = ppB.tile([NS, BG], fp32, tag="tps")
    nc.tensor.transpose(wsT_ps, wsum_p, ident[:BG, :BG])
    wsT = mp.tile([NS, BG], fp32)
    nc.scalar.copy(wsT, wsT_ps)
    wsum_b = mp.tile([NS, BPC], fp32)
    nc.vector.tensor_reduce(
        wsum_b, wsT.rearrange("p (b g) -> p b g", b=BPC),
        axis=X, op=Alu.add,
    )
    nc.vector.tensor_scalar(
        wsum_b, wsum_b, 1e-8, None, op0=Alu.add
    )
    recip = mp.tile([NS, BPC], fp32)
    nc.vector.reciprocal(recip, wsum_b)

    for b in range(BPC):
        if b < len(fc_tiles):
            f_c = fc_tiles[b]
        else:
            f_c = fbp.tile([128, KT, D], fp32, tag="f")
            nc.sync.dma_start(
                f_c, f_dr[b].rearrange("(kt p) d -> p kt d", p=128)
            )
        sp = ppA.tile([NS, D], fp32, tag="gps")
        for h, (h0, h1) in enumerate([(0, 512), (512, D)]):
            for kt in range(KT):
                nc.tensor.matmul(
                    sp[:, h0:h1],
                    s_ap(wT[:, kt, b, :]),
                    s_ap(f_c[:, kt, h0:h1]),
                    start=(kt == 0),
                    stop=(kt == KT - 1),
                )
        slot_sb = gsp.tile([NS, D], fp32, tag="slot")
        nc.scalar.activation(
            slot_sb, sp, Act.Copy, scale=recip[:, b:b + 1]
        )
        nc.sync.dma_start(out_dr[b], slot_sb)


def _get_nc(debug=False, reps=None):
    key = ("nc", debug, GRAM_DT, SLOT_DT, reps if reps is not None else REPS)
    if key not in _CACHE:
        _CACHE[key] = _build_nc(debug, reps=reps)
    return _CACHE[key]


def _consts():
    ident = np.eye(128, dtype=np.float32)
    E = np.zeros((NG, BG), dtype=np.float32)
    for i in range(BG):
        E[i // NG, i] = 1.0
    pcol = np.zeros((BG, 2), dtype=np.float32)
    pcol[:, 0] = (np.arange(BG) // NG) * N + (np.arange(BG) % NG) * 128
    pcol[:, 1] = np.arange(BG) % NG             # g
    crow = np.zeros((1, 128), dtype=np.float32)
    crow[0, :BG] = 65536.0                      # BIG (> any code)
    return ident, E, pcol, crow


def _make_in_maps(feats):
    ident, E, pcol, crow = _consts()
    return [
        {
            "features": feats[i * BPC:(i + 1) * BPC],
            "identity": ident,
            "econst": E,
            "pcol": pcol,
            "crow": crow,
        }
        for i in range(NC_CORES)
    ]


def kernel(features, batch_size=None, **_kw):
    from concourse import bass_utils

    nc = _get_nc(reps=1)
    feats = np.ascontiguousarray(np.asarray(features, dtype=np.float32))
    in_maps = _make_in_maps(feats)
    res = bass_utils.run_bass_kernel_spmd(
        nc, in_maps, core_ids=list(range(NC_CORES))
    )
    outs = [np.asarray(res.results[i]["slots"]) for i in range(NC_CORES)]
    return np.concatenate(outs, axis=0).astype(np.float32)
